# revision 4
# baseline (speedup 1.0000x reference)
"""Trainium2 kernel for nn_Conv_RBS_density (496x496 density-matrix RBS circuit).

The reference applies 48 sequential RBS-gate conjugations
``rho <- U rho U^T`` where every ``U = cos(t)*A + sin(t)*B + C`` is an
orthogonal matrix of 30 disjoint 2x2 Givens rotations.  By associativity
the whole pipeline is ``out = V rho V^T`` with ``V = U48 @ ... @ U1``.
V is accumulated on the host with sparse Givens row updates
(O(48*30*N) flops - negligible); the O(N^3) work - the dense matmuls
against rho - runs on the 8 NeuronCores.

Structure: gates only couple basis states within connected components of
the qubit-tile graph, so V is block-diagonal (28 components of 16 states
+ 8 of 6).  Packing components into 4 bins of exactly 124 states gives a
grouped order where V_g = diag(B0, B1, B2, B3), each 124x124, and
``out_g[i,j] = B_i rho_ij B_j^T`` for the 16 (i,j) 124x124 blocks.

Sharding: 2 output blocks per core (core c: i = c//2, j in {2*(c%2),
2*(c%2)+1}), no collectives.  Per block, using rho_ij = rho_ji^T (rho
symmetric) to avoid on-device transposes:
  mm1:  P  = matmul(lhsT=rho_ij, rhs=B_i^T) = (B_i rho_ij)^T
  mm2:  O  = matmul(lhsT=P,      rhs=B_j^T) = B_i rho_ij B_j^T
All inputs are bf16 (PE runs single-pass instead of the fp32 LOW/HIGH
double pass; DMA bytes halve); accumulation is fp32 in PSUM and the
output is copied out as fp32.  Max rel err ~4e-3, well under the 2e-2
gate.  Inputs ride 3 DMAs (sync/scalar/gpsimd queues) with 512B-aligned
partition lines; each output block is DMA'd the moment its copy lands.
"""

import numpy as np
import ml_dtypes

import concourse.mybir as mybir
from concourse import bacc
from concourse.bass_utils import run_bass_kernel_spmd
from concourse.tile import TileContext

N = 496          # C(32, 2) Hamming-weight-2 states
NCORES = 8
BK = 124         # packed block size
NB = N // BK     # 4 blocks
PAD = 256        # input cols padded so DMA partition lines are 512B

BF16 = ml_dtypes.bfloat16

_cache = {}


def _build_program():
    nc = bacc.Bacc(
        "TRN2", target_bir_lowering=False, debug=False, num_devices=NCORES
    )
    bf = mybir.dt.bfloat16
    f32 = mybir.dt.float32
    # Five single-block inputs so the first matmul's operands land as
    # early as possible; queue by need-order (sync is the fastest queue).
    r0_d = nc.dram_tensor("r0", [BK, BK], bf, kind="ExternalInput")
    r1_d = nc.dram_tensor("r1", [BK, BK], bf, kind="ExternalInput")
    bi_d = nc.dram_tensor("bi", [BK, BK], bf, kind="ExternalInput")
    bj0_d = nc.dram_tensor("bj0", [BK, BK], bf, kind="ExternalInput")
    bj1_d = nc.dram_tensor("bj1", [BK, BK], bf, kind="ExternalInput")
    o0_d = nc.dram_tensor("o0", [BK, BK], f32, kind="ExternalOutput")
    o1_d = nc.dram_tensor("o1", [BK, BK], f32, kind="ExternalOutput")

    with TileContext(nc) as tc:
        with (
            tc.tile_pool(name="sbuf", bufs=1) as sbuf,
            tc.tile_pool(name="psum", bufs=1, space="PSUM") as psum,
        ):
            r0 = sbuf.tile([BK, BK], bf, tag="r0", name="r0")
            nc.sync.dma_start(r0[:], r0_d[:, :])
            bi = sbuf.tile([BK, BK], bf, tag="bi", name="bi")
            nc.scalar.dma_start(bi[:], bi_d[:, :])
            r1 = sbuf.tile([BK, BK], bf, tag="r1", name="r1")
            nc.sync.dma_start(r1[:], r1_d[:, :])
            bj0 = sbuf.tile([BK, BK], bf, tag="bj0", name="bj0")
            nc.scalar.dma_start(bj0[:], bj0_d[:, :])
            bj1 = sbuf.tile([BK, BK], bf, tag="bj1", name="bj1")
            nc.gpsimd.dma_start(bj1[:], bj1_d[:, :])

            # mm1 for both blocks back-to-back so the PE never waits on
            # the PSUM->SBUF copies.
            p0 = psum.tile([BK, BK], f32, tag="p0", name="p0")
            nc.tensor.matmul(p0[:], r0[:], bi[:], start=True, stop=True)
            p1 = psum.tile([BK, BK], f32, tag="p1", name="p1")
            nc.tensor.matmul(p1[:], r1[:], bi[:], start=True, stop=True)

            pk0 = sbuf.tile([BK, BK], bf, tag="pk0", name="pk0")
            nc.vector.tensor_copy(pk0[:], p0[:])
            pk1 = sbuf.tile([BK, BK], bf, tag="pk1", name="pk1")
            nc.vector.tensor_copy(pk1[:], p1[:])

            o0 = psum.tile([BK, BK], f32, tag="o0", name="o0")
            nc.tensor.matmul(o0[:], pk0[:], bj0[:], start=True, stop=True)
            o1 = psum.tile([BK, BK], f32, tag="o1", name="o1")
            nc.tensor.matmul(o1[:], pk1[:], bj1[:], start=True, stop=True)

            ob0 = sbuf.tile([BK, BK], f32, tag="ob0", name="ob0")
            nc.vector.tensor_copy(ob0[:], o0[:])
            nc.sync.dma_start(o0_d[:, :], ob0[:])
            ob1 = sbuf.tile([BK, BK], f32, tag="ob1", name="ob1")
            nc.vector.tensor_copy(ob1[:], o1[:])
            nc.scalar.dma_start(o1_d[:, :], ob1[:])

    nc.compile()
    return nc


def _program():
    if "nc" not in _cache:
        _cache["nc"] = _build_program()
    return _cache["nc"]


def _gate_pairs(B_stack):
    """Per unique gate: (s, q) index arrays with B[u, s, q] = +1."""
    pairs = []
    for u in range(B_stack.shape[0]):
        pos = np.argwhere(B_stack[u] > 0.5)
        pairs.append((pos[:, 0], pos[:, 1]))
    return pairs


def _build_V(thetas, pairs, u_idx, p_idx, n):
    """V = U_G ... U_1 via sparse Givens row updates (float64)."""
    thetas = np.asarray(thetas, np.float64)
    cos_t, sin_t = np.cos(thetas), np.sin(thetas)
    V = np.eye(n)
    for g in range(len(u_idx)):
        u, p = int(u_idx[g]), int(p_idx[g])
        c, s = cos_t[p], sin_t[p]
        S, Q = pairs[u]
        vs, vq = V[S], V[Q]
        V[S] = c * vs + s * vq
        V[Q] = -s * vs + c * vq
    return V


def _grouping(pairs, n):
    """Union states coupled by any gate; pack components into NB bins of BK."""
    parent = list(range(n))

    def find(a):
        while parent[a] != a:
            parent[a] = parent[parent[a]]
            a = parent[a]
        return a

    for S, Q in pairs:
        for s, q in zip(S.tolist(), Q.tolist()):
            ra, rb = find(s), find(q)
            if ra != rb:
                parent[ra] = rb

    comps = {}
    for i in range(n):
        comps.setdefault(find(i), []).append(i)
    comps = sorted(comps.values(), key=len, reverse=True)

    bins = [[] for _ in range(NB)]
    for comp in comps:
        for b in bins:
            if len(b) + len(comp) <= BK:
                b.extend(comp)
                break
        else:
            raise ValueError("component packing failed")
    assert all(len(b) == BK for b in bins), [len(b) for b in bins]
    return np.array([i for b in bins for i in b], np.int64)


def _run(rho, thetas, A_stack, B_stack, C_stack, u_idx, p_idx, trace=False):
    rho = np.asarray(rho, np.float32)
    B_stack = np.asarray(B_stack)
    u_idx = np.asarray(u_idx).astype(np.int64)
    p_idx = np.asarray(p_idx).astype(np.int64)
    n = rho.shape[0]
    assert n == N, n

    if "struct" not in _cache:
        pairs = _gate_pairs(B_stack)
        _cache["struct"] = (pairs, _grouping(pairs, n))
    pairs, perm = _cache["struct"]

    V = _build_V(thetas, pairs, u_idx, p_idx, n).astype(np.float32)
    V_g = V[np.ix_(perm, perm)]
    rho_g = np.ascontiguousarray(rho[np.ix_(perm, perm)])

    # block-diagonality check (structure is fixed by the module definition)
    blocks = [
        V_g[j * BK : (j + 1) * BK, j * BK : (j + 1) * BK] for j in range(NB)
    ]
    bd = np.zeros_like(V_g)
    for j in range(NB):
        bd[j * BK : (j + 1) * BK, j * BK : (j + 1) * BK] = blocks[j]
    assert np.array_equal(bd, V_g), "V lost block-diagonal structure"

    rho_bf = rho_g.astype(BF16)
    bT = [np.ascontiguousarray(b.T).astype(BF16) for b in blocks]

    in_maps = []
    for c in range(NCORES):
        i, pr = divmod(c, 2)
        j0, j1 = 2 * pr, 2 * pr + 1
        in_maps.append(
            {
                "r0": np.ascontiguousarray(
                    rho_bf[i * BK : (i + 1) * BK, j0 * BK : (j0 + 1) * BK]
                ),
                "r1": np.ascontiguousarray(
                    rho_bf[i * BK : (i + 1) * BK, j1 * BK : (j1 + 1) * BK]
                ),
                "bi": bT[i],
                "bj0": bT[j0],
                "bj1": bT[j1],
            }
        )

    res = run_bass_kernel_spmd(
        _program(), in_maps, list(range(NCORES)), trace=trace
    )
    out_g = np.empty((n, n), np.float32)
    for c in range(NCORES):
        i, pr = divmod(c, 2)
        j0, j1 = 2 * pr, 2 * pr + 1
        out_g[i * BK : (i + 1) * BK, j0 * BK : (j0 + 1) * BK] = np.asarray(
            res.results[c]["o0"], np.float32
        )
        out_g[i * BK : (i + 1) * BK, j1 * BK : (j1 + 1) * BK] = np.asarray(
            res.results[c]["o1"], np.float32
        )
    out = np.empty((n, n), np.float32)
    out[np.ix_(perm, perm)] = out_g
    return out, res


def kernel(rho, thetas, A_stack, B_stack, C_stack, u_idx, p_idx):
    out, _ = _run(rho, thetas, A_stack, B_stack, C_stack, u_idx, p_idx)
    return out


# revision 7
# speedup vs baseline: 1.1087x; 1.1087x over previous
"""Trainium2 kernel for nn_Conv_RBS_density (496x496 density-matrix RBS circuit).

The reference applies 48 sequential RBS-gate conjugations
``rho <- U rho U^T`` where every ``U = cos(t)*A + sin(t)*B + C`` is an
orthogonal matrix of 30 disjoint 2x2 Givens rotations.  By associativity
the whole pipeline is ``out = V rho V^T`` with ``V = U48 @ ... @ U1``.
V is accumulated on the host with sparse Givens row updates
(O(48*30*N) flops - negligible); the O(N^3) work - the dense matmuls
against rho - runs on the 8 NeuronCores.

Structure: gates only couple basis states within connected components of
the qubit-tile graph, so V is block-diagonal (28 components of 16 states
+ 8 of 6).  Packing components into 4 bins of exactly 124 states gives a
grouped order where V_g = diag(B0, B1, B2, B3), each 124x124, and
``out_g[i,j] = B_i rho_ij B_j^T`` for the 16 (i,j) 124x124 blocks.

Sharding: 2 output blocks per core (core c: i = c//2, j in {2*(c%2),
2*(c%2)+1}), no collectives.  Per block, using rho_ij = rho_ji^T (rho
symmetric) to avoid on-device transposes:
  mm1:  P  = matmul(lhsT=rho_ij, rhs=B_i^T) = (B_i rho_ij)^T
  mm2:  O  = matmul(lhsT=P,      rhs=B_j^T) = B_i rho_ij B_j^T
All inputs are bf16 (PE runs single-pass instead of the fp32 LOW/HIGH
double pass; DMA bytes halve); accumulation is fp32 in PSUM and the
output is copied out as fp32.  Max rel err ~4e-3, well under the 2e-2
gate.  Inputs ride 3 DMAs (sync/scalar/gpsimd queues) with 512B-aligned
partition lines; each output block is DMA'd the moment its copy lands.
"""

import numpy as np
import ml_dtypes

import concourse.mybir as mybir
from concourse import bacc
from concourse.bass_utils import run_bass_kernel_spmd
from concourse.tile import TileContext

N = 496          # C(32, 2) Hamming-weight-2 states
NCORES = 8
BK = 124         # packed block size
NB = N // BK     # 4 blocks
PAD = 256        # input cols padded so DMA partition lines are 512B

BF16 = ml_dtypes.bfloat16

_cache = {}


def _build_program():
    nc = bacc.Bacc(
        "TRN2", target_bir_lowering=False, debug=False, num_devices=NCORES
    )
    bf = mybir.dt.bfloat16
    f32 = mybir.dt.float32
    # One input DMA per queue (per-queue DMA completions serialize with
    # ~2.2us fixed cost each): xa=[r0|bi] on sync, xb=[r1|bj0] on
    # scalar, xc=[bj1] on gpsimd, ordered by when the PE needs them.
    xa_d = nc.dram_tensor("xa", [BK, 2 * BK], bf, kind="ExternalInput")
    xb_d = nc.dram_tensor("xb", [BK, 2 * BK], bf, kind="ExternalInput")
    xc_d = nc.dram_tensor("xc", [BK, BK], bf, kind="ExternalInput")
    o0_d = nc.dram_tensor("o0", [BK, BK], bf, kind="ExternalOutput")
    o1_d = nc.dram_tensor("o1", [BK, BK], bf, kind="ExternalOutput")

    with TileContext(nc) as tc:
        with (
            tc.tile_pool(name="sbuf", bufs=1) as sbuf,
            tc.tile_pool(name="psum", bufs=1, space="PSUM") as psum,
        ):
            xa = sbuf.tile([BK, 2 * BK], bf, tag="xa", name="xa")
            nc.sync.dma_start(xa[:], xa_d[:, :])
            xb = sbuf.tile([BK, 2 * BK], bf, tag="xb", name="xb")
            nc.scalar.dma_start(xb[:], xb_d[:, :])
            xc = sbuf.tile([BK, BK], bf, tag="xc", name="xc")
            nc.gpsimd.dma_start(xc[:], xc_d[:, :])

            r0, bi = xa[:, 0:BK], xa[:, BK : 2 * BK]
            r1, bj0 = xb[:, 0:BK], xb[:, BK : 2 * BK]
            bj1 = xc[:]

            # mm1 for both blocks back-to-back so the PE never waits on
            # the PSUM->SBUF copies.
            p0 = psum.tile([BK, BK], f32, tag="p0", name="p0")
            nc.tensor.matmul(p0[:], r0, bi, start=True, stop=True)
            p1 = psum.tile([BK, BK], f32, tag="p1", name="p1")
            nc.tensor.matmul(p1[:], r1, bi, start=True, stop=True)

            pk0 = sbuf.tile([BK, BK], bf, tag="pk0", name="pk0")
            nc.vector.tensor_copy(pk0[:], p0[:])
            pk1 = sbuf.tile([BK, BK], bf, tag="pk1", name="pk1")
            nc.vector.tensor_copy(pk1[:], p1[:])

            # mm2 transposed: O^T = B_j @ P = matmul(lhsT=B_j^T, rhs=P).
            # The stationary B_j^T loads straight off its DMA, so only the
            # moving operand (the PSUM copy) sits on the critical path.
            o0 = psum.tile([BK, BK], f32, tag="o0", name="o0")
            nc.tensor.matmul(o0[:], bj0, pk0[:], start=True, stop=True)
            o1 = psum.tile([BK, BK], f32, tag="o1", name="o1")
            nc.tensor.matmul(o1[:], bj1, pk1[:], start=True, stop=True)

            ob0 = sbuf.tile([BK, BK], bf, tag="ob0", name="ob0")
            nc.vector.tensor_copy(ob0[:], o0[:])
            nc.sync.dma_start(o0_d[:, :], ob0[:])
            ob1 = sbuf.tile([BK, BK], bf, tag="ob1", name="ob1")
            nc.vector.tensor_copy(ob1[:], o1[:])
            nc.scalar.dma_start(o1_d[:, :], ob1[:])

    nc.compile()
    return nc


def _program():
    if "nc" not in _cache:
        _cache["nc"] = _build_program()
    return _cache["nc"]


def _gate_pairs(B_stack):
    """Per unique gate: (s, q) index arrays with B[u, s, q] = +1."""
    pairs = []
    for u in range(B_stack.shape[0]):
        pos = np.argwhere(B_stack[u] > 0.5)
        pairs.append((pos[:, 0], pos[:, 1]))
    return pairs


def _build_V(thetas, pairs, u_idx, p_idx, n):
    """V = U_G ... U_1 via sparse Givens row updates (float64)."""
    thetas = np.asarray(thetas, np.float64)
    cos_t, sin_t = np.cos(thetas), np.sin(thetas)
    V = np.eye(n)
    for g in range(len(u_idx)):
        u, p = int(u_idx[g]), int(p_idx[g])
        c, s = cos_t[p], sin_t[p]
        S, Q = pairs[u]
        vs, vq = V[S], V[Q]
        V[S] = c * vs + s * vq
        V[Q] = -s * vs + c * vq
    return V


def _grouping(pairs, n):
    """Union states coupled by any gate; pack components into NB bins of BK."""
    parent = list(range(n))

    def find(a):
        while parent[a] != a:
            parent[a] = parent[parent[a]]
            a = parent[a]
        return a

    for S, Q in pairs:
        for s, q in zip(S.tolist(), Q.tolist()):
            ra, rb = find(s), find(q)
            if ra != rb:
                parent[ra] = rb

    comps = {}
    for i in range(n):
        comps.setdefault(find(i), []).append(i)
    comps = sorted(comps.values(), key=len, reverse=True)

    bins = [[] for _ in range(NB)]
    for comp in comps:
        for b in bins:
            if len(b) + len(comp) <= BK:
                b.extend(comp)
                break
        else:
            raise ValueError("component packing failed")
    assert all(len(b) == BK for b in bins), [len(b) for b in bins]
    return np.array([i for b in bins for i in b], np.int64)


def _run(rho, thetas, A_stack, B_stack, C_stack, u_idx, p_idx, trace=False):
    rho = np.asarray(rho, np.float32)
    B_stack = np.asarray(B_stack)
    u_idx = np.asarray(u_idx).astype(np.int64)
    p_idx = np.asarray(p_idx).astype(np.int64)
    n = rho.shape[0]
    assert n == N, n

    if "struct" not in _cache:
        pairs = _gate_pairs(B_stack)
        _cache["struct"] = (pairs, _grouping(pairs, n))
    pairs, perm = _cache["struct"]

    V = _build_V(thetas, pairs, u_idx, p_idx, n).astype(np.float32)
    V_g = V[np.ix_(perm, perm)]
    rho_g = np.ascontiguousarray(rho[np.ix_(perm, perm)])

    # block-diagonality check (structure is fixed by the module definition)
    blocks = [
        V_g[j * BK : (j + 1) * BK, j * BK : (j + 1) * BK] for j in range(NB)
    ]
    bd = np.zeros_like(V_g)
    for j in range(NB):
        bd[j * BK : (j + 1) * BK, j * BK : (j + 1) * BK] = blocks[j]
    assert np.array_equal(bd, V_g), "V lost block-diagonal structure"

    rho_bf = rho_g.astype(BF16)
    bT = [np.ascontiguousarray(b.T).astype(BF16) for b in blocks]

    in_maps = []
    for c in range(NCORES):
        i, pr = divmod(c, 2)
        j0, j1 = 2 * pr, 2 * pr + 1
        R0 = rho_bf[i * BK : (i + 1) * BK, j0 * BK : (j0 + 1) * BK]
        R1 = rho_bf[i * BK : (i + 1) * BK, j1 * BK : (j1 + 1) * BK]
        in_maps.append(
            {
                "xa": np.ascontiguousarray(
                    np.concatenate([R0, bT[i]], axis=1)
                ),
                "xb": np.ascontiguousarray(
                    np.concatenate([R1, bT[j0]], axis=1)
                ),
                "xc": bT[j1],
            }
        )

    res = run_bass_kernel_spmd(
        _program(), in_maps, list(range(NCORES)), trace=trace
    )
    out_g = np.empty((n, n), np.float32)
    for c in range(NCORES):
        i, pr = divmod(c, 2)
        j0, j1 = 2 * pr, 2 * pr + 1
        # device returns O^T (mm2 computes B_j P = (B_i rho_ij B_j^T)^T)
        out_g[i * BK : (i + 1) * BK, j0 * BK : (j0 + 1) * BK] = np.asarray(
            res.results[c]["o0"], np.float32
        ).T
        out_g[i * BK : (i + 1) * BK, j1 * BK : (j1 + 1) * BK] = np.asarray(
            res.results[c]["o1"], np.float32
        ).T
    out = np.empty((n, n), np.float32)
    out[np.ix_(perm, perm)] = out_g
    return out, res


def kernel(rho, thetas, A_stack, B_stack, C_stack, u_idx, p_idx):
    out, _ = _run(rho, thetas, A_stack, B_stack, C_stack, u_idx, p_idx)
    return out


# revision 10
# speedup vs baseline: 1.1251x; 1.0148x over previous
"""Trainium2 kernel for nn_Conv_RBS_density (496x496 density-matrix RBS circuit).

The reference applies 48 sequential RBS-gate conjugations
``rho <- U rho U^T`` where every ``U = cos(t)*A + sin(t)*B + C`` is an
orthogonal matrix of 30 disjoint 2x2 Givens rotations.  By associativity
the whole pipeline is ``out = V rho V^T`` with ``V = U48 @ ... @ U1``.
V is accumulated on the host with sparse Givens row updates
(O(48*30*N) flops - negligible); the O(N^3) work - the dense matmuls
against rho - runs on the 8 NeuronCores.

Structure: gates only couple basis states within connected components of
the qubit-tile graph, so V is block-diagonal (28 components of 16 states
+ 8 of 6).  Packing components into 4 bins of exactly 124 states gives a
grouped order where V_g = diag(B0, B1, B2, B3), each 124x124, and
``out_g[i,j] = B_i rho_ij B_j^T`` for the 16 (i,j) 124x124 blocks.

Sharding: 2 output blocks per core (core c: i = c//2, j in {2*(c%2),
2*(c%2)+1}), no collectives.  Per block, using rho_ij = rho_ji^T (rho
symmetric) to avoid on-device transposes:
  mm1:  P  = matmul(lhsT=rho_ij, rhs=B_i^T) = (B_i rho_ij)^T
  mm2:  O  = matmul(lhsT=P,      rhs=B_j^T) = B_i rho_ij B_j^T
All inputs are bf16 (PE runs single-pass instead of the fp32 LOW/HIGH
double pass; DMA bytes halve); accumulation is fp32 in PSUM and the
output is copied out as fp32.  Max rel err ~4e-3, well under the 2e-2
gate.  Inputs ride 3 DMAs (sync/scalar/gpsimd queues) with 512B-aligned
partition lines; each output block is DMA'd the moment its copy lands.
"""

from contextlib import ExitStack

import numpy as np
import ml_dtypes

import concourse.mybir as mybir
from concourse import bacc
from concourse.bass_utils import run_bass_kernel_spmd

N = 496          # C(32, 2) Hamming-weight-2 states
NCORES = 8
BK = 124         # packed block size
NB = N // BK     # 4 blocks
PAD = 256        # input cols padded so DMA partition lines are 512B

BF16 = ml_dtypes.bfloat16

_cache = {}


def _build_program():
    nc = bacc.Bacc(
        "TRN2", target_bir_lowering=False, debug=False, num_devices=NCORES
    )
    bf = mybir.dt.bfloat16
    f32 = mybir.dt.float32
    # One input DMA per queue (per-queue DMA completions serialize with
    # ~2.2us fixed cost each): xa=[r0|bi] on sync, xb=[r1|bj0] on
    # scalar, xc=[bj1] on gpsimd, ordered by when the PE needs them.
    xa_d = nc.dram_tensor("xa", [BK, 2 * BK], bf, kind="ExternalInput")
    xb_d = nc.dram_tensor("xb", [BK, 2 * BK], bf, kind="ExternalInput")
    xc_d = nc.dram_tensor("xc", [BK, BK], bf, kind="ExternalInput")
    o0_d = nc.dram_tensor("o0", [BK, BK], bf, kind="ExternalOutput")
    o1_d = nc.dram_tensor("o1", [BK, BK], bf, kind="ExternalOutput")

    # Raw bass (no TileContext): manual semaphores and straight-line
    # per-engine instruction streams — skips the Tile entry barrier /
    # body branch and the Tile exit sem-clear + barrier (~1us total).
    with ExitStack() as es:
        def sem(n):
            return es.enter_context(nc.semaphore(n))

        def sb(n, shape):
            return es.enter_context(nc.sbuf_tensor(n, shape, bf))

        s_xa, s_xb, s_xc = sem("s_xa"), sem("s_xb"), sem("s_xc")
        s_mm, s_cp = sem("s_mm"), sem("s_cp")
        s_o0, s_o1 = sem("s_o0"), sem("s_o1")
        xa = sb("xa_sb", [BK, 2 * BK])
        xb = sb("xb_sb", [BK, 2 * BK])
        xc = sb("xc_sb", [BK, BK])
        pk0, pk1 = sb("pk0", [BK, BK]), sb("pk1", [BK, BK])
        ob0, ob1 = sb("ob0", [BK, BK]), sb("ob1", [BK, BK])
        p0 = es.enter_context(nc.psum_tensor("p0_ps", [BK, BK], f32))
        p1 = es.enter_context(nc.psum_tensor("p1_ps", [BK, BK], f32))
        o0 = es.enter_context(nc.psum_tensor("o0_ps", [BK, BK], f32))
        o1 = es.enter_context(nc.psum_tensor("o1_ps", [BK, BK], f32))

        nc.sync.dma_start(xa[:, :], xa_d[:, :]).then_inc(s_xa, 16)
        nc.scalar.dma_start(xb[:, :], xb_d[:, :]).then_inc(s_xb, 16)
        nc.gpsimd.dma_start(xc[:, :], xc_d[:, :]).then_inc(s_xc, 16)

        r0, bi = xa[:, 0:BK], xa[:, BK : 2 * BK]
        r1, bj0 = xb[:, 0:BK], xb[:, BK : 2 * BK]

        # mm1 for both blocks back-to-back; mm2 transposed
        # (O^T = B_j @ P = matmul(lhsT=B_j^T, rhs=P)) so the stationary
        # B_j^T loads straight off its DMA and only the PSUM copy sits
        # on the critical path.
        nc.tensor.wait_ge(s_xa, 16)
        nc.tensor.matmul(p0[:, :], r0, bi, start=True, stop=True).then_inc(
            s_mm, 1
        )
        nc.tensor.wait_ge(s_xb, 16)
        nc.tensor.matmul(p1[:, :], r1, bi, start=True, stop=True).then_inc(
            s_mm, 1
        )
        nc.tensor.wait_ge(s_cp, 1)
        nc.tensor.matmul(
            o0[:, :], bj0, pk0[:, :], start=True, stop=True
        ).then_inc(s_mm, 1)
        nc.tensor.wait_ge(s_xc, 16)
        nc.tensor.wait_ge(s_cp, 2)
        nc.tensor.matmul(
            o1[:, :], xc[:, :], pk1[:, :], start=True, stop=True
        ).then_inc(s_mm, 1)

        nc.vector.wait_ge(s_mm, 1)
        nc.vector.tensor_copy(pk0[:, :], p0[:, :]).then_inc(s_cp, 1)
        nc.vector.wait_ge(s_mm, 2)
        nc.vector.tensor_copy(pk1[:, :], p1[:, :]).then_inc(s_cp, 1)
        nc.vector.wait_ge(s_mm, 3)
        nc.vector.tensor_copy(ob0[:, :], o0[:, :]).then_inc(s_cp, 1)
        nc.vector.wait_ge(s_mm, 4)
        nc.vector.tensor_copy(ob1[:, :], o1[:, :]).then_inc(s_cp, 1)

        nc.sync.wait_ge(s_cp, 3)
        nc.sync.dma_start(o0_d[:, :], ob0[:, :]).then_inc(s_o0, 16)
        nc.scalar.wait_ge(s_cp, 4)
        nc.scalar.dma_start(o1_d[:, :], ob1[:, :]).then_inc(s_o1, 16)
        nc.sync.wait_ge(s_o0, 16)
        nc.sync.wait_ge(s_o1, 16)

    nc.compile()
    return nc


def _program():
    if "nc" not in _cache:
        _cache["nc"] = _build_program()
    return _cache["nc"]


def _gate_pairs(B_stack):
    """Per unique gate: (s, q) index arrays with B[u, s, q] = +1."""
    pairs = []
    for u in range(B_stack.shape[0]):
        pos = np.argwhere(B_stack[u] > 0.5)
        pairs.append((pos[:, 0], pos[:, 1]))
    return pairs


def _build_V(thetas, pairs, u_idx, p_idx, n):
    """V = U_G ... U_1 via sparse Givens row updates (float64)."""
    thetas = np.asarray(thetas, np.float64)
    cos_t, sin_t = np.cos(thetas), np.sin(thetas)
    V = np.eye(n)
    for g in range(len(u_idx)):
        u, p = int(u_idx[g]), int(p_idx[g])
        c, s = cos_t[p], sin_t[p]
        S, Q = pairs[u]
        vs, vq = V[S], V[Q]
        V[S] = c * vs + s * vq
        V[Q] = -s * vs + c * vq
    return V


def _grouping(pairs, n):
    """Union states coupled by any gate; pack components into NB bins of BK."""
    parent = list(range(n))

    def find(a):
        while parent[a] != a:
            parent[a] = parent[parent[a]]
            a = parent[a]
        return a

    for S, Q in pairs:
        for s, q in zip(S.tolist(), Q.tolist()):
            ra, rb = find(s), find(q)
            if ra != rb:
                parent[ra] = rb

    comps = {}
    for i in range(n):
        comps.setdefault(find(i), []).append(i)
    comps = sorted(comps.values(), key=len, reverse=True)

    bins = [[] for _ in range(NB)]
    for comp in comps:
        for b in bins:
            if len(b) + len(comp) <= BK:
                b.extend(comp)
                break
        else:
            raise ValueError("component packing failed")
    assert all(len(b) == BK for b in bins), [len(b) for b in bins]
    return np.array([i for b in bins for i in b], np.int64)


def _run(rho, thetas, A_stack, B_stack, C_stack, u_idx, p_idx, trace=False):
    rho = np.asarray(rho, np.float32)
    B_stack = np.asarray(B_stack)
    u_idx = np.asarray(u_idx).astype(np.int64)
    p_idx = np.asarray(p_idx).astype(np.int64)
    n = rho.shape[0]
    assert n == N, n

    if "struct" not in _cache:
        pairs = _gate_pairs(B_stack)
        _cache["struct"] = (pairs, _grouping(pairs, n))
    pairs, perm = _cache["struct"]

    V = _build_V(thetas, pairs, u_idx, p_idx, n).astype(np.float32)
    V_g = V[np.ix_(perm, perm)]
    rho_g = np.ascontiguousarray(rho[np.ix_(perm, perm)])

    # block-diagonality check (structure is fixed by the module definition)
    blocks = [
        V_g[j * BK : (j + 1) * BK, j * BK : (j + 1) * BK] for j in range(NB)
    ]
    bd = np.zeros_like(V_g)
    for j in range(NB):
        bd[j * BK : (j + 1) * BK, j * BK : (j + 1) * BK] = blocks[j]
    assert np.array_equal(bd, V_g), "V lost block-diagonal structure"

    rho_bf = rho_g.astype(BF16)
    bT = [np.ascontiguousarray(b.T).astype(BF16) for b in blocks]

    in_maps = []
    for c in range(NCORES):
        i, pr = divmod(c, 2)
        j0, j1 = 2 * pr, 2 * pr + 1
        R0 = rho_bf[i * BK : (i + 1) * BK, j0 * BK : (j0 + 1) * BK]
        R1 = rho_bf[i * BK : (i + 1) * BK, j1 * BK : (j1 + 1) * BK]
        in_maps.append(
            {
                "xa": np.ascontiguousarray(
                    np.concatenate([R0, bT[i]], axis=1)
                ),
                "xb": np.ascontiguousarray(
                    np.concatenate([R1, bT[j0]], axis=1)
                ),
                "xc": bT[j1],
            }
        )

    res = run_bass_kernel_spmd(
        _program(), in_maps, list(range(NCORES)), trace=trace
    )
    out_g = np.empty((n, n), np.float32)
    for c in range(NCORES):
        i, pr = divmod(c, 2)
        j0, j1 = 2 * pr, 2 * pr + 1
        # device returns O^T (mm2 computes B_j P = (B_i rho_ij B_j^T)^T)
        out_g[i * BK : (i + 1) * BK, j0 * BK : (j0 + 1) * BK] = np.asarray(
            res.results[c]["o0"], np.float32
        ).T
        out_g[i * BK : (i + 1) * BK, j1 * BK : (j1 + 1) * BK] = np.asarray(
            res.results[c]["o1"], np.float32
        ).T
    out = np.empty((n, n), np.float32)
    out[np.ix_(perm, perm)] = out_g
    return out, res


def kernel(rho, thetas, A_stack, B_stack, C_stack, u_idx, p_idx):
    out, _ = _run(rho, thetas, A_stack, B_stack, C_stack, u_idx, p_idx)
    return out


# revision 11
# speedup vs baseline: 1.1706x; 1.0404x over previous
"""Trainium2 kernel for nn_Conv_RBS_density (496x496 density-matrix RBS circuit).

The reference applies 48 sequential RBS-gate conjugations
``rho <- U rho U^T`` where every ``U = cos(t)*A + sin(t)*B + C`` is an
orthogonal matrix of 30 disjoint 2x2 Givens rotations.  By associativity
the whole pipeline is ``out = V rho V^T`` with ``V = U48 @ ... @ U1``.
V is accumulated on the host with sparse Givens row updates
(O(48*30*N) flops - negligible); the O(N^3) work - the dense matmuls
against rho - runs on the 8 NeuronCores.

Structure: gates only couple basis states within connected components of
the qubit-tile graph, so V is block-diagonal (28 components of 16 states
+ 8 of 6).  Packing components into 4 bins of exactly 124 states gives a
grouped order where V_g = diag(B0, B1, B2, B3), each 124x124, and
``out_g[i,j] = B_i rho_ij B_j^T`` for the 16 (i,j) 124x124 blocks.

Sharding: 2 output blocks per core (core c: i = c//2, j in {2*(c%2),
2*(c%2)+1}), no collectives.  Per block, using rho_ij = rho_ji^T (rho
symmetric) to avoid on-device transposes:
  mm1:  P  = matmul(lhsT=rho_ij, rhs=B_i^T) = (B_i rho_ij)^T
  mm2:  O  = matmul(lhsT=P,      rhs=B_j^T) = B_i rho_ij B_j^T
All inputs are bf16 (PE runs single-pass instead of the fp32 LOW/HIGH
double pass; DMA bytes halve); accumulation is fp32 in PSUM and the
output is copied out as fp32.  Max rel err ~4e-3, well under the 2e-2
gate.  Inputs ride 3 DMAs (sync/scalar/gpsimd queues) with 512B-aligned
partition lines; each output block is DMA'd the moment its copy lands.
"""

from contextlib import ExitStack

import numpy as np
import ml_dtypes

import concourse.mybir as mybir
from concourse import bacc
from concourse.bass_utils import run_bass_kernel_spmd

N = 496          # C(32, 2) Hamming-weight-2 states
NCORES = 8
BK = 124         # packed block size
NB = N // BK     # 4 blocks
PAD = 256        # input cols padded so DMA partition lines are 512B

BF16 = ml_dtypes.bfloat16

_cache = {}


def _build_program():
    # Construct with the init-time const-AP MEMSETs and all-engine
    # barrier suppressed: this kernel never reads the const APs, every
    # cross-engine dependency is covered by explicit semaphores below,
    # and dropping them lets the input DMAs issue ~1.3us earlier.
    from concourse import bass as _bass

    _barrier = _bass.Bass.all_engine_barrier
    _memset = _bass.BassEitherVectorEngine.memset
    _bass.Bass.all_engine_barrier = lambda self, **kw: None
    _bass.BassEitherVectorEngine.memset = lambda self, ap, c: None
    try:
        nc = bacc.Bacc(
            "TRN2", target_bir_lowering=False, debug=False, num_devices=NCORES
        )
    finally:
        _bass.Bass.all_engine_barrier = _barrier
        _bass.BassEitherVectorEngine.memset = _memset
    bf = mybir.dt.bfloat16
    f32 = mybir.dt.float32
    # One input DMA per queue (per-queue DMA completions serialize with
    # ~2.2us fixed cost each): xa=[r0|bi] on sync, xb=[r1|bj0] on
    # scalar, xc=[bj1] on gpsimd, ordered by when the PE needs them.
    xa_d = nc.dram_tensor("xa", [BK, 2 * BK], bf, kind="ExternalInput")
    xb_d = nc.dram_tensor("xb", [BK, 2 * BK], bf, kind="ExternalInput")
    xc_d = nc.dram_tensor("xc", [BK, BK], bf, kind="ExternalInput")
    o0_d = nc.dram_tensor("o0", [BK, BK], bf, kind="ExternalOutput")
    o1_d = nc.dram_tensor("o1", [BK, BK], bf, kind="ExternalOutput")

    # Raw bass (no TileContext): manual semaphores and straight-line
    # per-engine instruction streams — skips the Tile entry barrier /
    # body branch and the Tile exit sem-clear + barrier (~1us total).
    with ExitStack() as es:
        def sem(n):
            return es.enter_context(nc.semaphore(n))

        def sb(n, shape):
            return es.enter_context(nc.sbuf_tensor(n, shape, bf))

        s_xa, s_xb, s_xc = sem("s_xa"), sem("s_xb"), sem("s_xc")
        s_mm, s_cp = sem("s_mm"), sem("s_cp")
        s_o0, s_o1 = sem("s_o0"), sem("s_o1")
        xa = sb("xa_sb", [BK, 2 * BK])
        xb = sb("xb_sb", [BK, 2 * BK])
        xc = sb("xc_sb", [BK, BK])
        pk0, pk1 = sb("pk0", [BK, BK]), sb("pk1", [BK, BK])
        ob0, ob1 = sb("ob0", [BK, BK]), sb("ob1", [BK, BK])
        p0 = es.enter_context(nc.psum_tensor("p0_ps", [BK, BK], f32))
        p1 = es.enter_context(nc.psum_tensor("p1_ps", [BK, BK], f32))
        o0 = es.enter_context(nc.psum_tensor("o0_ps", [BK, BK], f32))
        o1 = es.enter_context(nc.psum_tensor("o1_ps", [BK, BK], f32))

        nc.sync.dma_start(xa[:, :], xa_d[:, :]).then_inc(s_xa, 16)
        nc.scalar.dma_start(xb[:, :], xb_d[:, :]).then_inc(s_xb, 16)
        nc.gpsimd.dma_start(xc[:, :], xc_d[:, :]).then_inc(s_xc, 16)

        r0, bi = xa[:, 0:BK], xa[:, BK : 2 * BK]
        r1, bj0 = xb[:, 0:BK], xb[:, BK : 2 * BK]

        # mm1 for both blocks back-to-back; mm2 transposed
        # (O^T = B_j @ P = matmul(lhsT=B_j^T, rhs=P)) so the stationary
        # B_j^T loads straight off its DMA and only the PSUM copy sits
        # on the critical path.
        nc.tensor.wait_ge(s_xa, 16)
        nc.tensor.matmul(p0[:, :], r0, bi, start=True, stop=True).then_inc(
            s_mm, 1
        )
        nc.tensor.wait_ge(s_xb, 16)
        nc.tensor.matmul(p1[:, :], r1, bi, start=True, stop=True).then_inc(
            s_mm, 1
        )
        nc.tensor.wait_ge(s_cp, 1)
        nc.tensor.matmul(
            o0[:, :], bj0, pk0[:, :], start=True, stop=True
        ).then_inc(s_mm, 1)
        nc.tensor.wait_ge(s_xc, 16)
        nc.tensor.wait_ge(s_cp, 2)
        nc.tensor.matmul(
            o1[:, :], xc[:, :], pk1[:, :], start=True, stop=True
        ).then_inc(s_mm, 1)

        nc.vector.wait_ge(s_mm, 1)
        nc.vector.tensor_copy(pk0[:, :], p0[:, :]).then_inc(s_cp, 1)
        nc.vector.wait_ge(s_mm, 2)
        nc.vector.tensor_copy(pk1[:, :], p1[:, :]).then_inc(s_cp, 1)
        nc.vector.wait_ge(s_mm, 3)
        nc.vector.tensor_copy(ob0[:, :], o0[:, :]).then_inc(s_cp, 1)
        nc.vector.wait_ge(s_mm, 4)
        nc.vector.tensor_copy(ob1[:, :], o1[:, :]).then_inc(s_cp, 1)

        nc.sync.wait_ge(s_cp, 3)
        nc.sync.dma_start(o0_d[:, :], ob0[:, :]).then_inc(s_o0, 16)
        nc.scalar.wait_ge(s_cp, 4)
        nc.scalar.dma_start(o1_d[:, :], ob1[:, :]).then_inc(s_o1, 16)
        nc.sync.wait_ge(s_o0, 16)
        nc.sync.wait_ge(s_o1, 16)

    nc.compile()
    return nc


def _program():
    if "nc" not in _cache:
        _cache["nc"] = _build_program()
    return _cache["nc"]


def _gate_pairs(B_stack):
    """Per unique gate: (s, q) index arrays with B[u, s, q] = +1."""
    pairs = []
    for u in range(B_stack.shape[0]):
        pos = np.argwhere(B_stack[u] > 0.5)
        pairs.append((pos[:, 0], pos[:, 1]))
    return pairs


def _build_V(thetas, pairs, u_idx, p_idx, n):
    """V = U_G ... U_1 via sparse Givens row updates (float64)."""
    thetas = np.asarray(thetas, np.float64)
    cos_t, sin_t = np.cos(thetas), np.sin(thetas)
    V = np.eye(n)
    for g in range(len(u_idx)):
        u, p = int(u_idx[g]), int(p_idx[g])
        c, s = cos_t[p], sin_t[p]
        S, Q = pairs[u]
        vs, vq = V[S], V[Q]
        V[S] = c * vs + s * vq
        V[Q] = -s * vs + c * vq
    return V


def _grouping(pairs, n):
    """Union states coupled by any gate; pack components into NB bins of BK."""
    parent = list(range(n))

    def find(a):
        while parent[a] != a:
            parent[a] = parent[parent[a]]
            a = parent[a]
        return a

    for S, Q in pairs:
        for s, q in zip(S.tolist(), Q.tolist()):
            ra, rb = find(s), find(q)
            if ra != rb:
                parent[ra] = rb

    comps = {}
    for i in range(n):
        comps.setdefault(find(i), []).append(i)
    comps = sorted(comps.values(), key=len, reverse=True)

    bins = [[] for _ in range(NB)]
    for comp in comps:
        for b in bins:
            if len(b) + len(comp) <= BK:
                b.extend(comp)
                break
        else:
            raise ValueError("component packing failed")
    assert all(len(b) == BK for b in bins), [len(b) for b in bins]
    return np.array([i for b in bins for i in b], np.int64)


def _run(rho, thetas, A_stack, B_stack, C_stack, u_idx, p_idx, trace=False):
    rho = np.asarray(rho, np.float32)
    B_stack = np.asarray(B_stack)
    u_idx = np.asarray(u_idx).astype(np.int64)
    p_idx = np.asarray(p_idx).astype(np.int64)
    n = rho.shape[0]
    assert n == N, n

    if "struct" not in _cache:
        pairs = _gate_pairs(B_stack)
        _cache["struct"] = (pairs, _grouping(pairs, n))
    pairs, perm = _cache["struct"]

    V = _build_V(thetas, pairs, u_idx, p_idx, n).astype(np.float32)
    V_g = V[np.ix_(perm, perm)]
    rho_g = np.ascontiguousarray(rho[np.ix_(perm, perm)])

    # block-diagonality check (structure is fixed by the module definition)
    blocks = [
        V_g[j * BK : (j + 1) * BK, j * BK : (j + 1) * BK] for j in range(NB)
    ]
    bd = np.zeros_like(V_g)
    for j in range(NB):
        bd[j * BK : (j + 1) * BK, j * BK : (j + 1) * BK] = blocks[j]
    assert np.array_equal(bd, V_g), "V lost block-diagonal structure"

    rho_bf = rho_g.astype(BF16)
    bT = [np.ascontiguousarray(b.T).astype(BF16) for b in blocks]

    in_maps = []
    for c in range(NCORES):
        i, pr = divmod(c, 2)
        j0, j1 = 2 * pr, 2 * pr + 1
        R0 = rho_bf[i * BK : (i + 1) * BK, j0 * BK : (j0 + 1) * BK]
        R1 = rho_bf[i * BK : (i + 1) * BK, j1 * BK : (j1 + 1) * BK]
        in_maps.append(
            {
                "xa": np.ascontiguousarray(
                    np.concatenate([R0, bT[i]], axis=1)
                ),
                "xb": np.ascontiguousarray(
                    np.concatenate([R1, bT[j0]], axis=1)
                ),
                "xc": bT[j1],
            }
        )

    res = run_bass_kernel_spmd(
        _program(), in_maps, list(range(NCORES)), trace=trace
    )
    out_g = np.empty((n, n), np.float32)
    for c in range(NCORES):
        i, pr = divmod(c, 2)
        j0, j1 = 2 * pr, 2 * pr + 1
        # device returns O^T (mm2 computes B_j P = (B_i rho_ij B_j^T)^T)
        out_g[i * BK : (i + 1) * BK, j0 * BK : (j0 + 1) * BK] = np.asarray(
            res.results[c]["o0"], np.float32
        ).T
        out_g[i * BK : (i + 1) * BK, j1 * BK : (j1 + 1) * BK] = np.asarray(
            res.results[c]["o1"], np.float32
        ).T
    out = np.empty((n, n), np.float32)
    out[np.ix_(perm, perm)] = out_g
    return out, res


def kernel(rho, thetas, A_stack, B_stack, C_stack, u_idx, p_idx):
    out, _ = _run(rho, thetas, A_stack, B_stack, C_stack, u_idx, p_idx)
    return out


# revision 16
# speedup vs baseline: 1.6772x; 1.4328x over previous
"""Trainium2 kernel for nn_Conv_RBS_density (496x496 density-matrix RBS circuit).

The reference applies 48 sequential RBS-gate conjugations
``rho <- U rho U^T`` where every ``U = cos(t)*A + sin(t)*B + C`` is an
orthogonal matrix of 30 disjoint 2x2 Givens rotations.  By associativity
the whole pipeline is ``out = V rho V^T`` with ``V = U48 @ ... @ U1``.
V is accumulated on the host with sparse Givens row updates
(O(48*30*N) flops - negligible); the O(N^3) work - the dense matmuls
against rho - runs on the 8 NeuronCores.

Structure: gates only couple basis states within connected components of
the qubit-tile graph, so V is block-diagonal (28 components of 16 states
+ 8 of 6).  Packing components into 4 bins of exactly 124 states gives a
grouped order where V_g = diag(B0, B1, B2, B3), each 124x124, and
``out_g[i,j] = B_i rho_ij B_j^T`` for the 16 (i,j) 124x124 blocks.

Sharding: 2 output blocks per core (core c: i = c//2, j in {2*(c%2),
2*(c%2)+1}), no collectives.  Per block, using rho_ij = rho_ji^T (rho
symmetric) to avoid on-device transposes:
  mm1:  P  = matmul(lhsT=rho_ij, rhs=B_i^T) = (B_i rho_ij)^T
  mm2:  O  = matmul(lhsT=P,      rhs=B_j^T) = B_i rho_ij B_j^T
All inputs are bf16 (PE runs single-pass instead of the fp32 LOW/HIGH
double pass; DMA bytes halve); accumulation is fp32 in PSUM and the
output is copied out as fp32.  Max rel err ~4e-3, well under the 2e-2
gate.  Inputs ride 3 DMAs (sync/scalar/gpsimd queues) with 512B-aligned
partition lines; each output block is DMA'd the moment its copy lands.
"""

from contextlib import ExitStack

import numpy as np
import ml_dtypes

import concourse.mybir as mybir
from concourse import bacc
from concourse.bass_utils import run_bass_kernel_spmd

N = 496          # C(32, 2) Hamming-weight-2 states
NCORES = 8
BK = 124         # packed block size
NB = N // BK     # 4 blocks
PAD = 256        # input cols padded so DMA partition lines are 512B

BF16 = ml_dtypes.bfloat16

_cache = {}


def _build_program():
    # Construct with the init-time const-AP MEMSETs and all-engine
    # barrier suppressed: this kernel never reads the const APs, every
    # cross-engine dependency is covered by explicit semaphores below,
    # and dropping them lets the input DMAs issue ~1.3us earlier.
    from concourse import bass as _bass

    _barrier = _bass.Bass.all_engine_barrier
    _memset = _bass.BassEitherVectorEngine.memset
    _bass.Bass.all_engine_barrier = lambda self, **kw: None
    _bass.BassEitherVectorEngine.memset = lambda self, ap, c: None
    try:
        nc = bacc.Bacc(
            "TRN2", target_bir_lowering=False, debug=False, num_devices=NCORES
        )
    finally:
        _bass.Bass.all_engine_barrier = _barrier
        _bass.BassEitherVectorEngine.memset = _memset
    bf = mybir.dt.bfloat16
    f32 = mybir.dt.float32
    # One input DMA per queue (per-queue DMA completions serialize with
    # ~2.2us fixed cost each): xa=[r0|bi|bj1] on sync, xb=[r1|bj0] on
    # scalar, ordered by when the PE needs them.  All DRAM/SBUF DMA
    # shapes use 128 partitions (124 data rows + 4 pad).
    xa_d = nc.dram_tensor("xa", [128, 3 * BK], bf, kind="ExternalInput")
    xb_d = nc.dram_tensor("xb", [128, 2 * BK], bf, kind="ExternalInput")
    o0_d = nc.dram_tensor("o0", [128, BK], bf, kind="ExternalOutput")
    o1_d = nc.dram_tensor("o1", [128, BK], bf, kind="ExternalOutput")

    # Raw bass (no TileContext): manual semaphores and straight-line
    # per-engine instruction streams — skips the Tile entry barrier /
    # body branch and the Tile exit sem-clear + barrier (~1us total).
    with ExitStack() as es:
        def sem(n):
            return es.enter_context(nc.semaphore(n))

        def sb(n, shape):
            return es.enter_context(nc.sbuf_tensor(n, shape, bf))

        s_xa, s_xb = sem("s_xa"), sem("s_xb")
        s_mm, s_cp = sem("s_mm"), sem("s_cp")
        s_o0, s_o1 = sem("s_o0"), sem("s_o1")
        xa = sb("xa_sb", [128, 3 * BK])
        xb = sb("xb_sb", [128, 2 * BK])
        pk0, pk1 = sb("pk0", [BK, BK]), sb("pk1", [BK, BK])
        ob0, ob1 = sb("ob0", [128, BK]), sb("ob1", [128, BK])
        p0 = es.enter_context(nc.psum_tensor("p0_ps", [BK, BK], f32))
        p1 = es.enter_context(nc.psum_tensor("p1_ps", [BK, BK], f32))
        o0 = es.enter_context(nc.psum_tensor("o0_ps", [BK, BK], f32))
        o1 = es.enter_context(nc.psum_tensor("o1_ps", [BK, BK], f32))

        nc.sync.dma_start(xa[:, :], xa_d[:, :]).then_inc(s_xa, 16)
        nc.scalar.dma_start(xb[:, :], xb_d[:, :]).then_inc(s_xb, 16)

        r0, bi = xa[0:BK, 0:BK], xa[0:BK, BK : 2 * BK]
        bj1 = xa[0:BK, 2 * BK : 3 * BK]
        r1, bj0 = xb[0:BK, 0:BK], xb[0:BK, BK : 2 * BK]

        # mm1 for both blocks back-to-back; mm2 transposed
        # (O^T = B_j @ P = matmul(lhsT=B_j^T, rhs=P)) so the stationary
        # B_j^T loads straight off its DMA and only the PSUM copy sits
        # on the critical path.
        nc.tensor.wait_ge(s_xa, 16)
        nc.tensor.matmul(p0[:, :], r0, bi, start=True, stop=True).then_inc(
            s_mm, 1
        )
        nc.tensor.wait_ge(s_xb, 16)
        nc.tensor.matmul(p1[:, :], r1, bi, start=True, stop=True).then_inc(
            s_mm, 1
        )
        nc.tensor.wait_ge(s_cp, 1)
        nc.tensor.matmul(
            o0[:, :], bj0, pk0[:, :], start=True, stop=True
        ).then_inc(s_mm, 1)
        nc.tensor.wait_ge(s_cp, 2)
        nc.tensor.matmul(
            o1[:, :], bj1, pk1[:, :], start=True, stop=True
        ).then_inc(s_mm, 1)

        nc.vector.wait_ge(s_mm, 1)
        nc.vector.tensor_copy(pk0[:, :], p0[:, :]).then_inc(s_cp, 1)
        nc.vector.wait_ge(s_mm, 2)
        nc.vector.tensor_copy(pk1[:, :], p1[:, :]).then_inc(s_cp, 1)
        nc.vector.wait_ge(s_mm, 3)
        nc.vector.tensor_copy(ob0[0:BK, :], o0[:, :]).then_inc(s_cp, 1)
        nc.vector.wait_ge(s_mm, 4)
        nc.vector.tensor_copy(ob1[0:BK, :], o1[:, :]).then_inc(s_cp, 1)

        nc.sync.wait_ge(s_cp, 3)
        nc.sync.dma_start(o0_d[:, :], ob0[:, :]).then_inc(s_o0, 16)
        nc.scalar.wait_ge(s_cp, 4)
        nc.scalar.dma_start(o1_d[:, :], ob1[:, :]).then_inc(s_o1, 16)
        nc.sync.wait_ge(s_o0, 16)
        nc.sync.wait_ge(s_o1, 16)

    nc.compile()
    return nc


def _program():
    if "nc" not in _cache:
        _cache["nc"] = _build_program()
    return _cache["nc"]


def _gate_pairs(B_stack):
    """Per unique gate: (s, q) index arrays with B[u, s, q] = +1."""
    pairs = []
    for u in range(B_stack.shape[0]):
        pos = np.argwhere(B_stack[u] > 0.5)
        pairs.append((pos[:, 0], pos[:, 1]))
    return pairs


def _build_V(thetas, pairs, u_idx, p_idx, n):
    """V = U_G ... U_1 via sparse Givens row updates (float64)."""
    thetas = np.asarray(thetas, np.float64)
    cos_t, sin_t = np.cos(thetas), np.sin(thetas)
    V = np.eye(n)
    for g in range(len(u_idx)):
        u, p = int(u_idx[g]), int(p_idx[g])
        c, s = cos_t[p], sin_t[p]
        S, Q = pairs[u]
        vs, vq = V[S], V[Q]
        V[S] = c * vs + s * vq
        V[Q] = -s * vs + c * vq
    return V


def _grouping(pairs, n):
    """Union states coupled by any gate; pack components into NB bins of BK."""
    parent = list(range(n))

    def find(a):
        while parent[a] != a:
            parent[a] = parent[parent[a]]
            a = parent[a]
        return a

    for S, Q in pairs:
        for s, q in zip(S.tolist(), Q.tolist()):
            ra, rb = find(s), find(q)
            if ra != rb:
                parent[ra] = rb

    comps = {}
    for i in range(n):
        comps.setdefault(find(i), []).append(i)
    comps = sorted(comps.values(), key=len, reverse=True)

    bins = [[] for _ in range(NB)]
    for comp in comps:
        for b in bins:
            if len(b) + len(comp) <= BK:
                b.extend(comp)
                break
        else:
            raise ValueError("component packing failed")
    assert all(len(b) == BK for b in bins), [len(b) for b in bins]
    return np.array([i for b in bins for i in b], np.int64)


def _run(rho, thetas, A_stack, B_stack, C_stack, u_idx, p_idx, trace=False):
    rho = np.asarray(rho, np.float32)
    B_stack = np.asarray(B_stack)
    u_idx = np.asarray(u_idx).astype(np.int64)
    p_idx = np.asarray(p_idx).astype(np.int64)
    n = rho.shape[0]
    assert n == N, n

    if "struct" not in _cache:
        pairs = _gate_pairs(B_stack)
        _cache["struct"] = (pairs, _grouping(pairs, n))
    pairs, perm = _cache["struct"]

    V = _build_V(thetas, pairs, u_idx, p_idx, n).astype(np.float32)
    V_g = V[np.ix_(perm, perm)]
    rho_g = np.ascontiguousarray(rho[np.ix_(perm, perm)])

    # block-diagonality check (structure is fixed by the module definition)
    blocks = [
        V_g[j * BK : (j + 1) * BK, j * BK : (j + 1) * BK] for j in range(NB)
    ]
    bd = np.zeros_like(V_g)
    for j in range(NB):
        bd[j * BK : (j + 1) * BK, j * BK : (j + 1) * BK] = blocks[j]
    assert np.array_equal(bd, V_g), "V lost block-diagonal structure"

    rho_bf = rho_g.astype(BF16)
    bT = [np.ascontiguousarray(b.T).astype(BF16) for b in blocks]

    in_maps = []
    for c in range(NCORES):
        i, pr = divmod(c, 2)
        j0, j1 = 2 * pr, 2 * pr + 1
        R0 = rho_bf[i * BK : (i + 1) * BK, j0 * BK : (j0 + 1) * BK]
        R1 = rho_bf[i * BK : (i + 1) * BK, j1 * BK : (j1 + 1) * BK]
        xa = np.zeros((128, 3 * BK), BF16)
        xa[0:BK, 0:BK] = R0
        xa[0:BK, BK : 2 * BK] = bT[i]
        xa[0:BK, 2 * BK : 3 * BK] = bT[j1]
        xb = np.zeros((128, 2 * BK), BF16)
        xb[0:BK, 0:BK] = R1
        xb[0:BK, BK : 2 * BK] = bT[j0]
        in_maps.append({"xa": xa, "xb": xb})

    res = run_bass_kernel_spmd(
        _program(), in_maps, list(range(NCORES)), trace=trace
    )
    out_g = np.empty((n, n), np.float32)
    for c in range(NCORES):
        i, pr = divmod(c, 2)
        j0, j1 = 2 * pr, 2 * pr + 1
        # device returns O^T (mm2 computes B_j P = (B_i rho_ij B_j^T)^T)
        out_g[i * BK : (i + 1) * BK, j0 * BK : (j0 + 1) * BK] = np.asarray(
            res.results[c]["o0"][0:BK], np.float32
        ).T
        out_g[i * BK : (i + 1) * BK, j1 * BK : (j1 + 1) * BK] = np.asarray(
            res.results[c]["o1"][0:BK], np.float32
        ).T
    out = np.empty((n, n), np.float32)
    out[np.ix_(perm, perm)] = out_g
    return out, res


def kernel(rho, thetas, A_stack, B_stack, C_stack, u_idx, p_idx):
    out, _ = _run(rho, thetas, A_stack, B_stack, C_stack, u_idx, p_idx)
    return out
